# revision 21
# baseline (speedup 1.0000x reference)
"""Trainium2 Bass kernel for nn_Phi_4578435137543 (scatter_memory).

Reference semantics: from a scalar x, compute the 5 Lagrange basis values
(and scaled 1st/2nd derivatives) of a degree-4 element, scatter them into
row `sample` of three (514, 128, 513) buffers, and return the stacked
`sample` slice -> (3, 1, 128, 513).  Only row `sample` of each buffer
affects the output, so the host shards exactly that row.

Sharding: the 513-node axis is split across 8 cores (65 nodes per core,
1-column overlap).  On-chip layout is [nodes=65 partitions, width=128 free]:
all node-dependent work happens in per-partition scalars, vectorized over a
20-wide free axis (3 value groups phi/dphi/ddphi + one indicator group), and
the buffer update is a single fused multiply-add over the [65, 3*128] tile.
"""

import numpy as np

N_WIDTH = 128
N_ORDER = 4
N_NODES = 513
N_CORES = 8
SEG = 64          # stride of per-core node ranges
P = 65            # partitions per core (64 + 1 overlap)
W3 = 3 * N_WIDTH  # concatenated phi|dphi|ddphi width segments
G = 15            # value groups: phi(5) | dphi(5) | ddphi(5)
AUXW = 2 + 5 * G  # aux width: x | off | 5 Horner coeff blocks of G
DELTA = 0.5 * N_ORDER / (N_NODES - 1)  # 1/256


def _lagrange_coeffs():
    """Monomial coefficients of the 5 Lagrange basis polynomials on nodes
    linspace(-1,1,5), plus 1st/2nd derivatives with the reference's 1/delta,
    1/delta^2 scaling folded in.  Returns [5, G] — row k is the t^k Horner
    block: [phi_j(5) | dphi_j(5) | ddphi_j(5) | inside-indicator(5)]."""
    nodes = np.linspace(-1.0, 1.0, 5)
    C = np.zeros((5, 5))
    D = np.zeros((5, 5))
    DD = np.zeros((5, 5))
    for j in range(5):
        p = np.poly1d([1.0])
        for m in range(5):
            if m != j:
                p *= np.poly1d([1.0, -nodes[m]]) / (nodes[j] - nodes[m])
        dp = p.deriv()
        ddp = dp.deriv()
        C[j, : len(p.c)] = p.c[::-1]
        D[j, : len(dp.c)] = dp.c[::-1]
        DD[j, : len(ddp.c)] = ddp.c[::-1]
    D /= DELTA
    DD /= DELTA ** 2
    blk = np.zeros((5, G), np.float32)   # [power k, group col]
    blk[:, 0:5] = C.T
    blk[:, 5:10] = D.T
    blk[:, 10:15] = DD.T
    return blk


_CACHE = {}


def _build():
    import concourse.bass as bass
    import concourse.bacc as bacc
    import concourse.mybir as mybir
    from concourse.tile import TileContext

    f32 = mybir.dt.float32
    Alu = mybir.AluOpType

    nc = bacc.Bacc("TRN2", target_bir_lowering=False)
    # Drop the 4 framework const-tile memsets (const-f32-0.0 etc.): they
    # serialize on Pool ahead of the entry barrier (~400ns) and their only
    # consumer is scalar.activation bias, which this kernel never uses.
    _blk = nc.m.functions[0].blocks[0]
    for _ins in [i for i in _blk.instructions
                 if i.__class__.__name__ == "InstMemset"
                 and "const-" in str(i.outs[0])]:
        _blk.instructions.remove(_ins)

    # aux: col 0 = x, col 1 = 64*core, cols 2: = Horner coeff blocks C4..C0
    aux = nc.dram_tensor("aux", [P, AUXW], f32, kind="ExternalInput")
    # bio: [phi | dphi | ddphi] row slices, transposed to [node, width]
    bio = nc.dram_tensor("bio", [P, W3], f32, kind="ExternalInput")
    out = nc.dram_tensor("out", [P, W3], f32, kind="ExternalOutput")

    with TileContext(nc) as tc:
        with tc.tile_pool(name="pool", bufs=1) as pool:
            ts = nc.vector.tensor_scalar
            tt = nc.vector.tensor_tensor
            stt = nc.vector.scalar_tensor_tensor

            at = pool.tile([P, AUXW], f32)
            nc.sync.dma_start(out=at[:, :], in_=aux[:, :])
            bt = pool.tile([P, W3], f32)
            nc.sync.dma_start(out=bt[:, :], in_=bio[:, :])
            xs = at[:, 0:1]
            offt = at[:, 1:2]

            def cblk(k):  # Horner block for power k (k=0..4)
                return at[:, 2 + (4 - k) * G: 2 + (5 - k) * G]

            # --- Pool: input-independent iotas + one x-dependent scalar ---
            thr = pool.tile([P, 127], f32)
            nc.gpsimd.iota(thr[:, :], [[1, 127]], base=1, channel_multiplier=0,
                           allow_small_or_imprecise_dtypes=True)
            nc.gpsimd.tensor_scalar(thr[:, :], thr[:, :], 1.0 / 128.0, None,
                                    Alu.mult)  # k/128, exact in f32
            pmj = pool.tile([P, G], f32)
            # value = p - j for each of 3 groups of 5
            nc.gpsimd.iota(pmj[:, :], [[0, 3], [-1, 5]], base=0,
                           channel_multiplier=1,
                           allow_small_or_imprecise_dtypes=True)
            xh = pool.tile([P, 1], f32)
            nc.gpsimd.tensor_scalar(xh[:, :], xs, 256.0, 1.0, Alu.mult,
                                    Alu.subtract)  # 256x - 1

            # --- DVE chain ---
            # floor(128 x) = sum_k [x >= k/128], exact in f32; the compare's
            # accum_out IS the row-sum, so no separate reduce.
            cmp = pool.tile([P, 127], f32)
            elf = pool.tile([P, 1], f32)
            ts(cmp[:, :], thr[:, :], xs, None, Alu.is_le, Alu.add,
               accum_out=elf[:, :])
            xt = pool.tile([P, 1], f32)
            stt(xt[:, :], elf[:, :], -2.0, xh[:, :], Alu.mult, Alu.add)
            nlo = pool.tile([P, 1], f32)
            stt(nlo[:, :], elf[:, :], 4.0, offt, Alu.mult, Alu.subtract)

            # outside = [p < nlo] + [p > nlo+4] (two fused DVE ops; Pool
            # cannot read per-partition Ptr scalars)
            o1 = pool.tile([P, 1], f32)
            outside = pool.tile([P, 1], f32)
            ts(o1[:, :], pmj[:, 0:1], nlo[:, :], None, Alu.is_lt)
            stt(outside[:, :], pmj[:, 4:5], nlo[:, :], o1[:, :],
                Alu.is_gt, Alu.add)

            # vals[:, g*5+j]: g=0 phi_j, g=1 dphi_j/delta, g=2 ddphi_j/d^2.
            # Estrin: vals = (C0 + C1 t) + t2*((C2 + C3 t) + C4 t2)
            t2 = pool.tile([P, 1], f32)
            tt(t2[:, :], xt[:, :], xt[:, :], Alu.mult)
            e1 = pool.tile([P, G], f32)
            stt(e1[:, :], cblk(1), xt[:, :], cblk(0), Alu.mult, Alu.add)
            e2 = pool.tile([P, G], f32)
            stt(e2[:, :], cblk(3), xt[:, :], cblk(2), Alu.mult, Alu.add)
            e3 = pool.tile([P, G], f32)
            stt(e3[:, :], cblk(4), t2[:, :], e2[:, :], Alu.mult, Alu.add)
            vals = pool.tile([P, G], f32)
            stt(vals[:, :], e3[:, :], t2[:, :], e1[:, :], Alu.mult, Alu.add)

            # Per group g: vprod[:,5g+j] = [p-j == nlo] * vals[:,5g+j], with
            # accum_out giving the group's scatter value directly.
            vprod = pool.tile([P, G], f32)
            vb4 = pool.tile([P, 4], f32)
            for g in range(3):
                s5 = slice(5 * g, 5 * g + 5)
                stt(vprod[:, s5], pmj[:, s5], nlo[:, :], vals[:, s5],
                    Alu.is_equal, Alu.mult, accum_out=vb4[:, g:g + 1])

            # bt = bt*outside + vb (vb is exactly 0 outside the window)
            bt3 = bt[:, :].rearrange("p (g w) -> p g w", w=N_WIDTH)
            vb3 = vb4[:, 0:3].unsqueeze(-1).broadcast_to((P, 3, N_WIDTH))
            stt(bt3, bt3, outside[:, :], vb3, Alu.mult, Alu.add)

            nc.sync.dma_start(out=out[:, :], in_=bt[:, :])

    nc.compile()
    return nc


def _make_in_maps(x, rows, coef):
    xv = np.float32(np.asarray(x).reshape(-1)[0])
    in_maps = []
    for c in range(N_CORES):
        sl = slice(SEG * c, SEG * c + P)
        aux = np.empty((P, AUXW), np.float32)
        aux[:, 0] = xv
        aux[:, 1] = np.float32(SEG * c)
        # Horner blocks in C4..C0 order to match cblk()
        aux[:, 2:] = coef[::-1].reshape(-1)
        bio = np.empty((P, W3), np.float32)
        for b in range(3):
            bio[:, b * N_WIDTH:(b + 1) * N_WIDTH] = rows[b][:, sl].T
        in_maps.append({"aux": aux, "bio": bio})
    return in_maps


def kernel(x, phi_ikp_inner, dphi_ikp_inner, ddphi_ikp_inner, sample, epoch,
           **_ignored):
    from concourse.bass_utils import run_bass_kernel_spmd

    if "nc" not in _CACHE:
        _CACHE["nc"] = _build()
        _CACHE["coef"] = _lagrange_coeffs()
    nc = _CACHE["nc"]

    s = int(np.asarray(sample))
    rows = [np.asarray(b[s], dtype=np.float32) for b in
            (phi_ikp_inner, dphi_ikp_inner, ddphi_ikp_inner)]  # each [128, 513]
    in_maps = _make_in_maps(x, rows, _CACHE["coef"])

    res = run_bass_kernel_spmd(nc, in_maps, core_ids=list(range(N_CORES))).results

    out = np.empty((3, 1, N_WIDTH, N_NODES), np.float32)
    for c in range(N_CORES):
        n = P if c == N_CORES - 1 else SEG
        o = res[c]["out"]
        for b in range(3):
            out[b, 0, :, SEG * c:SEG * c + n] = \
                o[:n, b * N_WIDTH:(b + 1) * N_WIDTH].T
    return out
